# revision 1
# baseline (speedup 1.0000x reference)
"""GroupAttention sparse-attention kernel for 8 trn2 NeuronCores.

Math (derived + numerically verified against the reference):
  - The mask keeps only tridiagonal scores -> softmax rows have >=1 finite
    entries at j=i+-1, or are fully uniform 1/S ("caseB" rows, where
    eos[i-1]=eos[i+1]=0).
  - neibor = v0 + (vBB-v0)*u u^T  (rank-1 over caseB flags u), overwritten on
    the 3 band diagonals with d_sup/d_main.
  - g[i,j] = exp(cum[j]-cum[i]) for j>i (sym.), diag d_main, +1e-9 off-diag,
    where cum = prefix-sum of ell=log(d_sup+1e-9).
  - scores use A~ = wq^T wk:  s[i,j] = xn_i A~ xn_j^T / 512.
SPMD: one program "compute rows 0..1023". core 2b -> batch b as-is;
core 2b+1 -> batch b with rows reversed (problem is reversal-covariant),
host un-reverses its output half. bq/bk/beta are zeros and gamma ones per the
problem spec, so they are folded away.
"""

import numpy as np
from contextlib import ExitStack

B, S, D = 4, 2048, 1024
NT = 8          # 128-row blocks per core (half of S/128)
HALF = S // 2

_cache = {}


def _build():
    import concourse.bass as bass
    import concourse.bacc as bacc
    import concourse.mybir as mybir
    from concourse.tile import TileContext

    f32 = mybir.dt.float32
    bf16 = mybir.dt.bfloat16
    i32 = mybir.dt.int32
    AF = mybir.ActivationFunctionType
    OP = mybir.AluOpType

    nc = bacc.Bacc("TRN2", target_bir_lowering=False)

    # ---------------- I/O ----------------
    x_in = nc.dram_tensor("x", [S, D], f32, kind="ExternalInput")
    eospad = nc.dram_tensor("eospad", [S + 2], i32, kind="ExternalInput")
    prior_t = nc.dram_tensor("prior", [1], f32, kind="ExternalInput")
    wq_in = nc.dram_tensor("wq", [D, D], f32, kind="ExternalInput")
    wk_in = nc.dram_tensor("wk", [D, D], f32, kind="ExternalInput")
    lt_in = nc.dram_tensor("lt128", [128, 128], f32, kind="ExternalInput")
    wup_in = nc.dram_tensor("wup", [128, 128], f32, kind="ExternalInput")
    wlo_in = nc.dram_tensor("wlo", [128, 128], f32, kind="ExternalInput")
    ones_in = nc.dram_tensor("onesb", [128, 1], bf16, kind="ExternalInput")
    zeros_in = nc.dram_tensor("zerosf", [16], f32, kind="ExternalInput")
    out_nb = nc.dram_tensor("out_nb", [HALF, S], f32, kind="ExternalOutput")
    out_g = nc.dram_tensor("out_g", [HALF, S], f32, kind="ExternalOutput")

    C_SQ9 = float(np.sqrt(np.float32(1e-9)))                    # sqrt(1e-9)
    C_SBB = float(np.sqrt(np.float32((1.0 / S) ** 2 + 1e-9)))   # caseB diag sqrt

    with TileContext(nc) as tc, ExitStack() as ctx:
        # ---------------- pools (whole-kernel lifetime) ----------------
        consts = ctx.enter_context(tc.tile_pool(name="consts", bufs=1))
        vec = ctx.enter_context(tc.tile_pool(name="vec", bufs=28))
        col = ctx.enter_context(tc.tile_pool(name="col", bufs=10))
        at_pool = ctx.enter_context(tc.tile_pool(name="atp", bufs=1))
        xnt_pool = ctx.enter_context(tc.tile_pool(name="xntp", bufs=1))
        psA = ctx.enter_context(tc.tile_pool(name="psA", bufs=2, space="PSUM"))
        psB = ctx.enter_context(tc.tile_pool(name="psB", bufs=1, space="PSUM"))
        dram = ctx.enter_context(tc.tile_pool(name="dram", bufs=1, space="DRAM"))

        # ---------------- consts into SBUF ----------------
        lt128 = consts.tile([128, 128], f32)
        nc.sync.dma_start(out=lt128, in_=lt_in[:, :])
        wup = consts.tile([128, 128], f32)
        nc.sync.dma_start(out=wup, in_=wup_in[:, :])
        wlo = consts.tile([128, 128], f32)
        nc.sync.dma_start(out=wlo, in_=wlo_in[:, :])
        ones_b = consts.tile([128, 1], bf16)
        nc.sync.dma_start(out=ones_b, in_=ones_in[:, :])
        pr_col = consts.tile([128, 1], f32)
        nc.sync.dma_start(
            out=pr_col,
            in_=bass.AP(tensor=prior_t[:].tensor, offset=prior_t[:].offset, ap=[[0, 128], [1, 1]]),
        )
        omp_col = consts.tile([128, 1], f32)  # 1 - prior
        nc.vector.tensor_scalar(omp_col, pr_col, -1.0, 1.0, OP.mult, OP.add)
        # v0 / vBB / (vBB-v0) as [128,1] broadcast columns
        v0_col = consts.tile([128, 1], f32)
        nc.vector.tensor_scalar(v0_col, omp_col, C_SQ9, None, OP.mult)
        nc.vector.tensor_tensor(v0_col, v0_col, pr_col, OP.add)
        vbb_col = consts.tile([128, 1], f32)
        nc.vector.tensor_scalar(vbb_col, omp_col, C_SBB, None, OP.mult)
        nc.vector.tensor_tensor(vbb_col, vbb_col, pr_col, OP.add)
        dv_col = consts.tile([128, 1], f32)  # vBB - v0
        nc.vector.tensor_tensor(dv_col, vbb_col, v0_col, OP.subtract)
        neg9 = consts.tile([128, 16], f32)
        nc.vector.memset(neg9, -1.0e9)
        # register const bias columns used by activation(bias=float)
        for ci, cval in enumerate((0.0, 1e-9, 1e-5)):
            cc = consts.tile([128, 1], f32, name=f"cc{ci}", tag=f"cc{ci}")
            nc.vector.memset(cc, cval)
            nc.const_aps.aps[(f32, cval)] = cc[:, :]

        # ---------------- DRAM scratch ----------------
        xb_d = dram.tile([S, D], bf16)          # normalized x, bf16
        snext_d = dram.tile([S], f32)
        sprev_d = dram.tile([S], f32)
        cum_d = dram.tile([S], f32)
        uscl_d = dram.tile([S], f32)            # (vBB-v0)*u
        u_d = dram.tile([S], f32)
        dsup_d = dram.tile([S + 1], f32)        # [0]=0, [1+i]=d_sup[i]
        dmain_d = dram.tile([S], f32)

        # ============ phase 1: LN+cast x ; A~^T = wk^T wq (bf16) ============
        with ExitStack() as p1:
            wpool = p1.enter_context(tc.tile_pool(name="wpool", bufs=1))
            xpool = p1.enter_context(tc.tile_pool(name="xpool", bufs=3))
            xbpool = p1.enter_context(tc.tile_pool(name="xbpool", bufs=3))
            stpool = p1.enter_context(tc.tile_pool(name="stpool", bufs=4))

            wqb = wpool.tile([128, 8, D], bf16)
            nc.gpsimd.dma_start(
                out=wqb[:, :, :], in_=wq_in[:, :].rearrange("(t p) e -> p t e", p=128)
            )
            wkb = wpool.tile([128, 8, D], bf16)
            nc.gpsimd.dma_start(
                out=wkb[:, :, :], in_=wk_in[:, :].rearrange("(t p) e -> p t e", p=128)
            )

            at_sb = at_pool.tile([128, 8, D], bf16)  # AT[p,ft,e] = A~^T[f,e]
            for ft in range(8):
                ps = psA.tile([128, D], f32)
                for dt in range(8):
                    for c in range(2):
                        nc.tensor.matmul(
                            ps[:, c * 512:(c + 1) * 512],
                            wkb[:, dt, ft * 128:(ft + 1) * 128],
                            wqb[:, dt, c * 512:(c + 1) * 512],
                            start=(dt == 0),
                            stop=(dt == 7),
                        )
                if ft % 2 == 0:
                    nc.vector.tensor_copy(out=at_sb[:, ft, :], in_=ps[:, :])
                else:
                    nc.scalar.copy(out=at_sb[:, ft, :], in_=ps[:, :])

            # --- LN per 128-row tile, write bf16 normalized x to DRAM ---
            for it in range(16):
                xt = xpool.tile([128, D], f32)
                nc.sync.dma_start(out=xt, in_=x_in[it * 128:(it + 1) * 128, :])
                stats = stpool.tile([128, 2, 6], f32)
                nc.vector.bn_stats(out=stats[:, 0, :], in_=xt[:, 0:512])
                nc.vector.bn_stats(out=stats[:, 1, :], in_=xt[:, 512:1024])
                mv = stpool.tile([128, 2], f32)
                nc.vector.bn_aggr(out=mv, in_=stats)
                # rstd = exp(-0.5*ln(var+1e-5))
                rstd = stpool.tile([128, 1], f32)
                nc.scalar.activation(rstd, mv[:, 1:2], AF.Ln, bias=1e-5)
                nc.scalar.activation(rstd, rstd, AF.Exp, scale=-0.5)
                xbt = xbpool.tile([128, D], bf16)
                nc.vector.tensor_scalar(
                    xbt, xt, mv[:, 0:1], rstd, OP.subtract, OP.mult
                )
                nc.sync.dma_start(out=xb_d[it * 128:(it + 1) * 128, :], in_=xbt)

        # ============ phase 2: transpose; z; band dot-products ============
        xnt = xnt_pool.tile([128, 8, S], bf16)   # xnt[p,ft,i] = xn[i, ft*128+p]
        for ft in range(8):
            nc.sync.dma_start(
                out=xnt[:, ft, :], in_=xb_d[:, ft * 128:(ft + 1) * 128],
                transpose=True,
            )

        with ExitStack() as p2:
            zpool = p2.enter_context(tc.tile_pool(name="zpool", bufs=2))
            p1pool = p2.enter_context(tc.tile_pool(name="p1pool", bufs=2))
            p2pool = p2.enter_context(tc.tile_pool(name="p2pool", bufs=8))
            rows = p2.enter_context(tc.tile_pool(name="rows", bufs=2))

            ps_n = psB.tile([1, S], f32, tag="psrow", name="ps_n")          # s_next accumulator
            p2tiles = []
            for et in range(8):
                zb = zpool.tile([128, S], bf16)
                for half in range(2):
                    ps = psA.tile([128, 1024], f32)
                    for ft in range(8):
                        for c in range(2):
                            off = half * 1024 + c * 512
                            nc.tensor.matmul(
                                ps[:, c * 512:(c + 1) * 512],
                                at_sb[:, ft, et * 128:(et + 1) * 128],
                                xnt[:, ft, off:off + 512],
                                start=(ft == 0),
                                stop=(ft == 7),
                            )
                    eng_copy = nc.vector.tensor_copy if half == 0 else nc.scalar.copy
                    eng_copy(out=zb[:, half * 1024:(half + 1) * 1024], in_=ps)
                # products
                pt1 = p1pool.tile([128, S], bf16)
                nc.vector.tensor_tensor(
                    pt1[:, 0:S - 1], xnt[:, et, 0:S - 1], zb[:, 1:S], OP.mult
                )
                pt2 = p2pool.tile([128, S], bf16)
                nc.vector.tensor_tensor(
                    pt2[:, 1:S], xnt[:, et, 1:S], zb[:, 0:S - 1], OP.mult
                )
                p2tiles.append(pt2)
                for c in range(4):
                    nc.tensor.matmul(
                        ps_n[0:1, c * 512:(c + 1) * 512],
                        ones_b,
                        pt1[:, c * 512:(c + 1) * 512],
                        start=(et == 0),
                        stop=(et == 7),
                    )
            row_n = rows.tile([1, S], f32)
            nc.scalar.mul(row_n, ps_n[0:1, :], 1.0 / 512.0)
            nc.sync.dma_start(out=snext_d[:], in_=row_n)

            ps_p = psB.tile([1, S], f32, tag="psrow", name="ps_p")
            for et in range(8):
                for c in range(4):
                    nc.tensor.matmul(
                        ps_p[0:1, c * 512:(c + 1) * 512],
                        ones_b,
                        p2tiles[et][:, c * 512:(c + 1) * 512],
                        start=(et == 0),
                        stop=(et == 7),
                    )
            row_p = rows.tile([1, S], f32)
            nc.scalar.mul(row_p, ps_p[0:1, :], 1.0 / 512.0)
            nc.sync.dma_start(out=sprev_d[:], in_=row_p)

        # ============ phase 3: band math in [128,16] layout ============
        def v16():
            return vec.tile([128, 16], f32, tag="v16", name="v16")

        def rd16(dtensor, off):  # dram vec [off:off+2048] -> [128,16] row-major
            return dtensor[off:off + S].rearrange("(p c) -> p c", c=16)

        sn = v16()
        nc.sync.dma_start(out=sn, in_=rd16(snext_d, 0))
        sp = v16()
        nc.sync.dma_start(out=sp, in_=rd16(sprev_d, 0))
        em_i = vec.tile([128, 16], i32)
        nc.sync.dma_start(out=em_i, in_=rd16(eospad[:], 1))
        hn_i = vec.tile([128, 16], i32)
        nc.sync.dma_start(out=hn_i, in_=rd16(eospad[:], 2))
        hp_i = vec.tile([128, 16], i32)
        nc.sync.dma_start(out=hp_i, in_=rd16(eospad[:], 0))
        hn = v16()
        nc.vector.tensor_copy(out=hn, in_=hn_i)
        hp = v16()
        nc.vector.tensor_copy(out=hp, in_=hp_i)

        sne = v16()
        nc.vector.select(sne, hn_i, sn, neg9)
        spe = v16()
        nc.vector.select(spe, hp_i, sp, neg9)
        m = v16()
        nc.vector.tensor_tensor(m, sne, spe, OP.max)
        en = v16()
        nc.vector.tensor_tensor(en, sne, m, OP.subtract)
        nc.scalar.activation(en, en, AF.Exp)
        ep = v16()
        nc.vector.tensor_tensor(ep, spe, m, OP.subtract)
        nc.scalar.activation(ep, ep, AF.Exp)
        zs = v16()
        nc.vector.tensor_tensor(zs, en, ep, OP.add)
        rz = v16()
        nc.vector.reciprocal(rz, zs)
        nn = v16()
        nc.vector.tensor_tensor(nn, en, rz, OP.mult)
        npv = v16()
        nc.vector.tensor_tensor(npv, ep, rz, OP.mult)
        # caseB flag u = (1-hn)*(1-hp); blend N with uniform 1/S
        t1 = v16()
        nc.vector.tensor_scalar(t1, hn, -1.0, 1.0, OP.mult, OP.add)
        t2 = v16()
        nc.vector.tensor_scalar(t2, hp, -1.0, 1.0, OP.mult, OP.add)
        cb = v16()
        nc.vector.tensor_tensor(cb, t1, t2, OP.mult)
        omcb = v16()
        nc.vector.tensor_scalar(omcb, cb, -1.0, 1.0, OP.mult, OP.add)
        cbS = v16()
        nc.vector.tensor_scalar(cbS, cb, 1.0 / S, None, OP.mult)
        for nv in (nn, npv):
            nc.vector.tensor_tensor(nv, nv, omcb, OP.mult)
            nc.vector.tensor_tensor(nv, nv, cbS, OP.add)
        # Np shifted by +1 (value at i+1)
        npsh = v16()
        nc.vector.memset(npsh, 0.0)
        nc.vector.tensor_copy(out=npsh[:, 0:15], in_=npv[:, 1:16])
        nc.sync.dma_start(out=npsh[0:127, 15:16], in_=npv[1:128, 0:1])
        msup = v16()
        nc.vector.tensor_tensor(msup, nn, npsh, OP.mult)
        # d_sup = prior + (1-prior)*exp(0.5*ln(msup+1e-9))
        dsup = v16()
        nc.scalar.activation(dsup, msup, AF.Ln, bias=1e-9)
        nc.scalar.activation(dsup, dsup, AF.Exp, scale=0.5)
        nc.vector.tensor_scalar(dsup, dsup, omp_col, pr_col, OP.mult, OP.add)
        # d_main = prior + (1-prior)*(c1 + (c2-c1)*cb)
        dmain = v16()
        nc.vector.tensor_scalar(dmain, cb, C_SBB - C_SQ9, C_SQ9, OP.mult, OP.add)
        nc.vector.tensor_scalar(dmain, dmain, omp_col, pr_col, OP.mult, OP.add)
        # ell, prefix sums
        ell = v16()
        nc.scalar.activation(ell, dsup, AF.Ln, bias=1e-9)
        zv16 = v16()
        nc.vector.memset(zv16, 0.0)
        incl = v16()
        nc.vector.tensor_tensor_scan(incl, ell, zv16, 0.0, OP.add, OP.add)
        excl = v16()
        nc.vector.tensor_tensor(excl, incl, ell, OP.subtract)
        ps_c = psA.tile([128, 1024], f32, tag="ps", name="ps_c")
        nc.tensor.matmul(
            ps_c[:, 0:1], lt128, incl[:, 15:16], start=True, stop=True
        )
        cp_col = col.tile([128, 1], f32)
        nc.vector.tensor_copy(out=cp_col, in_=ps_c[:, 0:1])
        cum = v16()
        nc.vector.tensor_scalar(cum, excl, cp_col, None, OP.add)
        uscl = v16()
        nc.vector.tensor_scalar(uscl, cb, dv_col, None, OP.mult)

        def wr16(dtensor, off, src):
            nc.sync.dma_start(
                out=dtensor[off:off + S].rearrange("(p c) -> p c", c=16), in_=src
            )

        wr16(cum_d, 0, cum)
        wr16(uscl_d, 0, uscl)
        wr16(u_d, 0, cb)
        wr16(dsup_d, 1, dsup)
        wr16(dmain_d, 0, dmain)

        # ============ phase 4: outputs ============
        with ExitStack() as p3:
            bcast = p3.enter_context(tc.tile_pool(name="bcast", bufs=1))
            outp = p3.enter_context(tc.tile_pool(name="outp", bufs=3))
            gwin = p3.enter_context(tc.tile_pool(name="gwin", bufs=6))
            colp = p3.enter_context(tc.tile_pool(name="colp", bufs=1))

            urow = bcast.tile([128, S], f32)
            nc.sync.dma_start(
                out=urow,
                in_=bass.AP(tensor=uscl_d[:].tensor, offset=uscl_d[:].offset,
                            ap=[[0, 128], [1, S]]),
            )
            cumrow = bcast.tile([128, S], f32)
            nc.sync.dma_start(
                out=cumrow,
                in_=bass.AP(tensor=cum_d[:].tensor, offset=cum_d[:].offset,
                            ap=[[0, 128], [1, S]]),
            )
            ucols = colp.tile([128, 8], f32)
            nc.sync.dma_start(
                out=ucols, in_=u_d[0:HALF].rearrange("(t p) -> p t", p=128)
            )
            cumcols = colp.tile([128, 8], f32)
            nc.sync.dma_start(
                out=cumcols, in_=cum_d[0:HALF].rearrange("(t p) -> p t", p=128)
            )

            for t in range(NT):
                r0 = t * 128
                nb = outp.tile([128, S], f32)
                nc.vector.tensor_scalar(
                    nb, urow, ucols[:, t:t + 1], v0_col, OP.mult, OP.add
                )
                nc.sync.dma_start(out=out_nb[r0:r0 + 128, :], in_=nb)

                g = outp.tile([128, S], f32)
                nc.vector.tensor_scalar(
                    g, cumrow, cumcols[:, t:t + 1], None, OP.subtract
                )
                if t > 0:
                    nc.scalar.activation(g[:, 0:r0], g[:, 0:r0], AF.Exp, scale=-1.0)
                nc.scalar.activation(
                    g[:, r0 + 128:S], g[:, r0 + 128:S], AF.Exp, scale=1.0
                )
                w = g[:, r0:r0 + 128]
                c1t = gwin.tile([128, 128], f32)
                nc.vector.tensor_scalar(c1t, w, 0.5, None, OP.min)
                e1 = gwin.tile([128, 128], f32)
                nc.scalar.activation(e1, c1t, AF.Exp)
                c2t = gwin.tile([128, 128], f32)
                nc.vector.tensor_scalar(c2t, w, -0.5, None, OP.max)
                e2 = gwin.tile([128, 128], f32)
                nc.scalar.activation(e2, c2t, AF.Exp, scale=-1.0)
                nc.vector.tensor_tensor(e1, e1, wup, OP.mult)
                nc.vector.tensor_tensor(e2, e2, wlo, OP.mult)
                nc.vector.tensor_tensor(w, e1, e2, OP.add)
                nc.gpsimd.tensor_scalar(g, g, 1.0e-9, None, OP.add)
                nc.sync.dma_start(out=out_g[r0:r0 + 128, :], in_=g)

            # band diagonals straight into DRAM (strided DRAM->DRAM copies)
            def diag_ap(dt, offset, count):
                return bass.AP(tensor=dt[:, :].tensor, offset=dt[:, :].offset + offset,
                               ap=[[S + 1, count]])

            nc.sync.dma_start(out=diag_ap(out_nb, 1, HALF), in_=dsup_d[1:1 + HALF])
            nc.sync.dma_start(out=diag_ap(out_nb, S, HALF - 1),
                              in_=dsup_d[1:HALF])
            nc.sync.dma_start(out=diag_ap(out_nb, 0, HALF), in_=dmain_d[0:HALF])
            nc.sync.dma_start(out=diag_ap(out_g, 0, HALF), in_=dmain_d[0:HALF])

    nc.compile()
    return nc


def _consts():
    k = np.arange(128)
    lt = (k[:, None] < k[None, :]).astype(np.float32)       # lt[k,p]=k<p
    wup = (k[None, :] > k[:, None]).astype(np.float32)      # wup[p,w]=w>p
    wlo = (k[None, :] < k[:, None]).astype(np.float32)
    import ml_dtypes
    ones = np.ones((128, 1), dtype=ml_dtypes.bfloat16)
    zer = np.zeros(16, np.float32)
    return lt, wup, wlo, ones, zer


def kernel(context, eos_mask, prior, wq, bq, wk, bk, gamma, beta):
    from concourse.bass_utils import run_bass_kernel_spmd

    if "nc" not in _cache:
        _cache["nc"] = _build()
    nc = _cache["nc"]

    context = np.asarray(context, np.float32)
    eos_mask = np.asarray(eos_mask, np.int32)
    prior = np.asarray(prior, np.float32)
    wq = np.asarray(wq, np.float32)
    wk = np.asarray(wk, np.float32)
    lt, wup, wlo, ones, zer = _consts()

    in_maps = []
    for c in range(8):
        b, h = c // 2, c % 2
        x = context[b] if h == 0 else context[b][::-1]
        eo = eos_mask[b] if h == 0 else eos_mask[b][::-1]
        eop = np.zeros(S + 2, np.int32)
        eop[1:S + 1] = eo
        in_maps.append({
            "x": np.ascontiguousarray(x),
            "eospad": eop,
            "prior": prior,
            "wq": wq, "wk": wk,
            "lt128": lt, "wup": wup, "wlo": wlo,
            "onesb": ones, "zerosf": zer,
        })

    bkr = run_bass_kernel_spmd(nc, in_maps, core_ids=list(range(8)))
    _cache["last_bkr"] = bkr

    g_out = np.empty((B, S, S), np.float32)
    nb_out = np.empty((B, S, S), np.float32)
    for c in range(8):
        b, h = c // 2, c % 2
        rg = bkr.results[c]["out_g"]
        rn = bkr.results[c]["out_nb"]
        if h == 0:
            g_out[b, :HALF] = rg
            nb_out[b, :HALF] = rn
        else:
            g_out[b, HALF:] = rg[::-1, ::-1]
            nb_out[b, HALF:] = rn[::-1, ::-1]
    return g_out, nb_out



# revision 10
# speedup vs baseline: 1.9052x; 1.9052x over previous
"""GroupAttention sparse-attention kernel for 8 trn2 NeuronCores.

Math (derived + numerically verified against the reference):
  - The mask keeps only tridiagonal scores -> softmax rows have >=1 finite
    entries at j=i+-1, or are fully uniform 1/S ("caseB" rows, where
    eos[i-1]=eos[i+1]=0).
  - neibor = v0 + (vBB-v0)*u u^T  (rank-1 over caseB flags u), overwritten on
    the 3 band diagonals with d_sup/d_main.
  - g[i,j] = exp(cum[j]-cum[i]) for j>i (sym.), diag d_main, +1e-9 off-diag,
    where cum = prefix-sum of ell=log(d_sup+1e-9).
  - scores use A~ = wq^T wk:  s[i,j] = xn_i A~ xn_j^T / 512.
SPMD: one program "compute rows 0..1023". core 2b -> batch b as-is;
core 2b+1 -> batch b with rows reversed (problem is reversal-covariant),
host un-reverses its output half. bq/bk/beta are zeros and gamma ones per the
problem spec, so they are folded away.
"""

import numpy as np
from contextlib import ExitStack

B, S, D = 4, 2048, 1024
NT = 8          # 128-row blocks per core (half of S/128)
HALF = S // 2

_cache = {}


def _build():
    import concourse.bass as bass
    import concourse.bacc as bacc
    import concourse.mybir as mybir
    from concourse.tile import TileContext

    f32 = mybir.dt.float32
    bf16 = mybir.dt.bfloat16
    i32 = mybir.dt.int32
    AF = mybir.ActivationFunctionType
    OP = mybir.AluOpType

    nc = bacc.Bacc("TRN2", target_bir_lowering=False)

    # ---------------- I/O ----------------
    x_in = nc.dram_tensor("x", [S, D], f32, kind="ExternalInput")
    eospad = nc.dram_tensor("eospad", [S + 2], i32, kind="ExternalInput")
    prior_t = nc.dram_tensor("prior", [1], f32, kind="ExternalInput")
    wq_in = nc.dram_tensor("wq", [D, D], f32, kind="ExternalInput")
    wk_in = nc.dram_tensor("wk", [D, D], f32, kind="ExternalInput")
    lt_in = nc.dram_tensor("lt128", [128, 128], f32, kind="ExternalInput")
    wup_in = nc.dram_tensor("wup", [128, 128], f32, kind="ExternalInput")
    wlo_in = nc.dram_tensor("wlo", [128, 128], f32, kind="ExternalInput")
    ones_in = nc.dram_tensor("onesb", [128, 1], bf16, kind="ExternalInput")
    zeros_in = nc.dram_tensor("zerosf", [16], f32, kind="ExternalInput")
    out_nb = nc.dram_tensor("out_nb", [HALF, S], f32, kind="ExternalOutput")
    out_g = nc.dram_tensor("out_g", [HALF, S], f32, kind="ExternalOutput")

    C_SQ9 = float(np.sqrt(np.float32(1e-9)))                    # sqrt(1e-9)
    C_SBB = float(np.sqrt(np.float32((1.0 / S) ** 2 + 1e-9)))   # caseB diag sqrt

    with TileContext(nc) as tc, ExitStack() as ctx:
        # ---------------- pools (whole-kernel lifetime) ----------------
        consts = ctx.enter_context(tc.tile_pool(name="consts", bufs=1))
        vec = ctx.enter_context(tc.tile_pool(name="vec", bufs=28))
        col = ctx.enter_context(tc.tile_pool(name="col", bufs=10))
        at_pool = ctx.enter_context(tc.tile_pool(name="atp", bufs=1))
        xnt_pool = ctx.enter_context(tc.tile_pool(name="xntp", bufs=1))
        psA = ctx.enter_context(tc.tile_pool(name="psA", bufs=2, space="PSUM"))
        psB = ctx.enter_context(tc.tile_pool(name="psB", bufs=1, space="PSUM"))
        dram = ctx.enter_context(tc.tile_pool(name="dram", bufs=1, space="DRAM"))

        # ---------------- consts into SBUF ----------------
        lt128 = consts.tile([128, 128], f32)
        nc.sync.dma_start(out=lt128, in_=lt_in[:, :])
        ones_b = consts.tile([128, 1], bf16)
        nc.sync.dma_start(out=ones_b, in_=ones_in[:, :])
        pr_col = consts.tile([128, 1], f32)
        nc.sync.dma_start(
            out=pr_col,
            in_=bass.AP(tensor=prior_t[:].tensor, offset=prior_t[:].offset, ap=[[0, 128], [1, 1]]),
        )
        omp_col = consts.tile([128, 1], f32)  # 1 - prior
        nc.vector.tensor_scalar(omp_col, pr_col, -1.0, 1.0, OP.mult, OP.add)
        # v0 / vBB / (vBB-v0) as [128,1] broadcast columns
        v0_col = consts.tile([128, 1], f32)
        nc.vector.tensor_scalar(v0_col, omp_col, C_SQ9, None, OP.mult)
        nc.vector.tensor_tensor(v0_col, v0_col, pr_col, OP.add)
        vbb_col = consts.tile([128, 1], f32)
        nc.vector.tensor_scalar(vbb_col, omp_col, C_SBB, None, OP.mult)
        nc.vector.tensor_tensor(vbb_col, vbb_col, pr_col, OP.add)
        dv_col = consts.tile([128, 1], f32)  # vBB - v0
        nc.vector.tensor_tensor(dv_col, vbb_col, v0_col, OP.subtract)
        neg9 = consts.tile([128, 16], f32)
        nc.vector.memset(neg9, -1.0e9)
        # register const bias columns used by activation(bias=float)
        for ci, cval in enumerate((0.0, 1e-9, 1e-5)):
            cc = consts.tile([128, 1], f32, name=f"cc{ci}", tag=f"cc{ci}")
            nc.vector.memset(cc, cval)
            nc.const_aps.aps[(f32, cval)] = cc[:, :]

        # ---------------- DRAM scratch ----------------
        xb_d = dram.tile([S, D], bf16)          # normalized x, bf16
        snext_d = dram.tile([S], f32)
        sprev_d = dram.tile([S], f32)
        cum_d = dram.tile([S], f32)
        uscl_d = dram.tile([S], f32)            # (vBB-v0)*u
        u_d = dram.tile([S], f32)
        dsup_d = dram.tile([S + 1], f32)        # [0]=0, [1+i]=d_sup[i]
        dmain_d = dram.tile([S], f32)

        # ============ phase 1: LN+cast x ; A~^T = wk^T wq (bf16) ============
        with ExitStack() as p1:
            wpool = p1.enter_context(tc.tile_pool(name="wpool", bufs=1))
            xpool = p1.enter_context(tc.tile_pool(name="xpool", bufs=3))
            xbpool = p1.enter_context(tc.tile_pool(name="xbpool", bufs=3))
            stpool = p1.enter_context(tc.tile_pool(name="stpool", bufs=4))

            wqb = wpool.tile([128, 8, D], bf16)
            nc.gpsimd.dma_start(
                out=wqb[:, :, :], in_=wq_in[:, :].rearrange("(t p) e -> p t e", p=128)
            )
            wkb = wpool.tile([128, 8, D], bf16)
            nc.gpsimd.dma_start(
                out=wkb[:, :, :], in_=wk_in[:, :].rearrange("(t p) e -> p t e", p=128)
            )

            at_sb = at_pool.tile([128, 8, D], bf16)  # AT[p,ft,e] = A~^T[f,e]
            for ft in range(8):
                ps = psA.tile([128, D], f32)
                for dt in range(8):
                    for c in range(2):
                        nc.tensor.matmul(
                            ps[:, c * 512:(c + 1) * 512],
                            wkb[:, dt, ft * 128:(ft + 1) * 128],
                            wqb[:, dt, c * 512:(c + 1) * 512],
                            start=(dt == 0),
                            stop=(dt == 7),
                        )
                if ft % 2 == 0:
                    nc.vector.tensor_copy(out=at_sb[:, ft, :], in_=ps[:, :])
                else:
                    nc.scalar.copy(out=at_sb[:, ft, :], in_=ps[:, :])

            # --- LN per 128-row tile, write bf16 normalized x to DRAM ---
            for it in range(16):
                xt = xpool.tile([128, D], f32)
                nc.sync.dma_start(out=xt, in_=x_in[it * 128:(it + 1) * 128, :])
                stats = stpool.tile([128, 2, 6], f32)
                nc.vector.bn_stats(out=stats[:, 0, :], in_=xt[:, 0:512])
                nc.vector.bn_stats(out=stats[:, 1, :], in_=xt[:, 512:1024])
                mv = stpool.tile([128, 2], f32)
                nc.vector.bn_aggr(out=mv, in_=stats)
                # rstd = 1/sqrt(var+1e-5); Sqrt keeps one ACT table set resident
                sdt = stpool.tile([128, 1], f32)
                nc.scalar.activation(sdt, mv[:, 1:2], AF.Sqrt, bias=1e-5)
                rstd = stpool.tile([128, 1], f32)
                nc.vector.reciprocal(rstd, sdt)
                xbt = xbpool.tile([128, D], bf16)
                nc.vector.tensor_scalar(
                    xbt, xt, mv[:, 0:1], rstd, OP.subtract, OP.mult
                )
                nc.sync.dma_start(out=xb_d[it * 128:(it + 1) * 128, :], in_=xbt)

        # ============ phase 2: transpose; z; band dot-products ============
        xnt = xnt_pool.tile([128, 8, S], bf16)   # xnt[p,ft,i] = xn[i, ft*128+p]
        for ft in range(8):
            nc.sync.dma_start(
                out=xnt[:, ft, :], in_=xb_d[:, ft * 128:(ft + 1) * 128],
                transpose=True,
            )

        with ExitStack() as p2:
            zpool = p2.enter_context(tc.tile_pool(name="zpool", bufs=2))
            p1pool = p2.enter_context(tc.tile_pool(name="p1pool", bufs=2))
            p2pool = p2.enter_context(tc.tile_pool(name="p2pool", bufs=8))
            rows = p2.enter_context(tc.tile_pool(name="rows", bufs=2))

            ps_n = psB.tile([1, S], f32, tag="psrow", name="ps_n")          # s_next accumulator
            p2tiles = []
            for et in range(8):
                zb = zpool.tile([128, S], bf16)
                for half in range(2):
                    ps = psA.tile([128, 1024], f32)
                    for ft in range(8):
                        for c in range(2):
                            off = half * 1024 + c * 512
                            nc.tensor.matmul(
                                ps[:, c * 512:(c + 1) * 512],
                                at_sb[:, ft, et * 128:(et + 1) * 128],
                                xnt[:, ft, off:off + 512],
                                start=(ft == 0),
                                stop=(ft == 7),
                            )
                    eng_copy = nc.vector.tensor_copy if half == 0 else nc.scalar.copy
                    eng_copy(out=zb[:, half * 1024:(half + 1) * 1024], in_=ps)
                # products
                pt1 = p1pool.tile([128, S], bf16)
                nc.vector.tensor_tensor(
                    pt1[:, 0:S - 1], xnt[:, et, 0:S - 1], zb[:, 1:S], OP.mult
                )
                pt2 = p2pool.tile([128, S], bf16)
                nc.vector.tensor_tensor(
                    pt2[:, 1:S], xnt[:, et, 1:S], zb[:, 0:S - 1], OP.mult
                )
                p2tiles.append(pt2)
                for c in range(4):
                    nc.tensor.matmul(
                        ps_n[0:1, c * 512:(c + 1) * 512],
                        ones_b,
                        pt1[:, c * 512:(c + 1) * 512],
                        start=(et == 0),
                        stop=(et == 7),
                    )
            row_n = rows.tile([1, S], f32)
            nc.scalar.mul(row_n, ps_n[0:1, :], 1.0 / 512.0)
            nc.sync.dma_start(out=snext_d[:], in_=row_n)

            ps_p = psB.tile([1, S], f32, tag="psrow", name="ps_p")
            for et in range(8):
                for c in range(4):
                    nc.tensor.matmul(
                        ps_p[0:1, c * 512:(c + 1) * 512],
                        ones_b,
                        p2tiles[et][:, c * 512:(c + 1) * 512],
                        start=(et == 0),
                        stop=(et == 7),
                    )
            row_p = rows.tile([1, S], f32)
            nc.scalar.mul(row_p, ps_p[0:1, :], 1.0 / 512.0)
            nc.sync.dma_start(out=sprev_d[:], in_=row_p)

        # ============ phase 3: band math in [128,16] layout ============
        def v16():
            return vec.tile([128, 16], f32, tag="v16", name="v16")

        def rd16(dtensor, off):  # dram vec [off:off+2048] -> [128,16] row-major
            return dtensor[off:off + S].rearrange("(p c) -> p c", c=16)

        sn = v16()
        nc.sync.dma_start(out=sn, in_=rd16(snext_d, 0))
        sp = v16()
        nc.sync.dma_start(out=sp, in_=rd16(sprev_d, 0))
        hn_i = vec.tile([128, 16], i32)
        nc.sync.dma_start(out=hn_i, in_=rd16(eospad[:], 2))
        hp_i = vec.tile([128, 16], i32)
        nc.sync.dma_start(out=hp_i, in_=rd16(eospad[:], 0))
        hn = v16()
        nc.vector.tensor_copy(out=hn, in_=hn_i)
        hp = v16()
        nc.vector.tensor_copy(out=hp, in_=hp_i)

        sne = v16()
        nc.vector.select(sne, hn_i, sn, neg9)
        spe = v16()
        nc.vector.select(spe, hp_i, sp, neg9)
        m = v16()
        nc.vector.tensor_tensor(m, sne, spe, OP.max)
        en = v16()
        nc.vector.tensor_tensor(en, sne, m, OP.subtract)
        nc.scalar.activation(en, en, AF.Exp)
        ep = v16()
        nc.vector.tensor_tensor(ep, spe, m, OP.subtract)
        nc.scalar.activation(ep, ep, AF.Exp)
        zs = v16()
        nc.vector.tensor_tensor(zs, en, ep, OP.add)
        rz = v16()
        nc.vector.reciprocal(rz, zs)
        nn = v16()
        nc.vector.tensor_tensor(nn, en, rz, OP.mult)
        npv = v16()
        nc.vector.tensor_tensor(npv, ep, rz, OP.mult)
        # caseB flag u = (1-hn)*(1-hp); blend N with uniform 1/S
        t1 = v16()
        nc.vector.tensor_scalar(t1, hn, -1.0, 1.0, OP.mult, OP.add)
        t2 = v16()
        nc.vector.tensor_scalar(t2, hp, -1.0, 1.0, OP.mult, OP.add)
        cb = v16()
        nc.vector.tensor_tensor(cb, t1, t2, OP.mult)
        omcb = v16()
        nc.vector.tensor_scalar(omcb, cb, -1.0, 1.0, OP.mult, OP.add)
        cbS = v16()
        nc.vector.tensor_scalar(cbS, cb, 1.0 / S, None, OP.mult)
        for nv in (nn, npv):
            nc.vector.tensor_tensor(nv, nv, omcb, OP.mult)
            nc.vector.tensor_tensor(nv, nv, cbS, OP.add)
        # Np shifted by +1 (value at i+1)
        npsh = v16()
        nc.vector.memset(npsh, 0.0)
        nc.vector.tensor_copy(out=npsh[:, 0:15], in_=npv[:, 1:16])
        nc.sync.dma_start(out=npsh[0:127, 15:16], in_=npv[1:128, 0:1])
        msup = v16()
        nc.vector.tensor_tensor(msup, nn, npsh, OP.mult)
        # d_sup = prior + (1-prior)*sqrt(msup+1e-9)
        dsup = v16()
        nc.scalar.activation(dsup, msup, AF.Sqrt, bias=1e-9)
        nc.vector.tensor_scalar(dsup, dsup, omp_col, pr_col, OP.mult, OP.add)
        # d_main = prior + (1-prior)*(c1 + (c2-c1)*cb)
        dmain = v16()
        nc.vector.tensor_scalar(dmain, cb, C_SBB - C_SQ9, C_SQ9, OP.mult, OP.add)
        nc.vector.tensor_scalar(dmain, dmain, omp_col, pr_col, OP.mult, OP.add)
        # ell, prefix sums
        ell = v16()
        nc.scalar.activation(ell, dsup, AF.Ln, bias=1e-9)
        zv16 = v16()
        nc.vector.memset(zv16, 0.0)
        incl = v16()
        nc.vector.tensor_tensor_scan(incl, ell, zv16, 0.0, OP.add, OP.add)
        excl = v16()
        nc.vector.tensor_tensor(excl, incl, ell, OP.subtract)
        ps_c = psA.tile([128, 1024], f32, tag="ps", name="ps_c")
        nc.tensor.matmul(
            ps_c[:, 0:1], lt128, incl[:, 15:16], start=True, stop=True
        )
        cp_col = col.tile([128, 1], f32)
        nc.vector.tensor_copy(out=cp_col, in_=ps_c[:, 0:1])
        cum = v16()
        nc.vector.tensor_scalar(cum, excl, cp_col, None, OP.add)
        uscl = v16()
        nc.vector.tensor_scalar(uscl, cb, dv_col, None, OP.mult)

        def wr16(dtensor, off, src):
            nc.sync.dma_start(
                out=dtensor[off:off + S].rearrange("(p c) -> p c", c=16), in_=src
            )

        wr16(cum_d, 0, cum)
        wr16(uscl_d, 0, uscl)
        wr16(u_d, 0, cb)
        wr16(dsup_d, 1, dsup)
        wr16(dmain_d, 0, dmain)

        # ============ phase 4: outputs ============
        with ExitStack() as p3:
            bcast = p3.enter_context(tc.tile_pool(name="bcast", bufs=1))
            outp = p3.enter_context(tc.tile_pool(name="outp", bufs=3))
            gwin = p3.enter_context(tc.tile_pool(name="gwin", bufs=4))
            colp = p3.enter_context(tc.tile_pool(name="colp", bufs=1))

            urow = bcast.tile([128, S], f32)
            nc.sync.dma_start(
                out=urow,
                in_=bass.AP(tensor=uscl_d[:].tensor, offset=uscl_d[:].offset,
                            ap=[[0, 128], [1, S]]),
            )
            cumrow = bcast.tile([128, S], f32)
            nc.sync.dma_start(
                out=cumrow,
                in_=bass.AP(tensor=cum_d[:].tensor, offset=cum_d[:].offset,
                            ap=[[0, 128], [1, S]]),
            )
            ucols = colp.tile([128, 8], f32)
            nc.sync.dma_start(
                out=ucols, in_=u_d[0:HALF].rearrange("(t p) -> p t", p=128)
            )
            cumcols = colp.tile([128, 8], f32)
            nc.sync.dma_start(
                out=cumcols, in_=cum_d[0:HALF].rearrange("(t p) -> p t", p=128)
            )
            ncc = colp.tile([128, 8], f32)  # -cum_i (bias for right/upper exp)
            nc.vector.tensor_scalar(ncc, cumcols, -1.0, None, OP.mult)

            for t in range(NT):
                r0 = t * 128
                nb = outp.tile([128, S], f32)
                nc.vector.tensor_scalar(
                    nb, urow, ucols[:, t:t + 1], v0_col, OP.mult, OP.add
                )
                nc.sync.dma_start(out=out_nb[r0:r0 + 128, :], in_=nb)

                # g = exp(-|cum_j - cum_i|), built on ACT: per-segment the sign
                # of (cum_j - cum_i) is known, so Exp(scale*cumrow + bias) is
                # safe; the 128-wide diagonal block uses min(exp(d), exp(-d)).
                # Diagonal patched later via diag DMA; reference's +1e-9 is
                # dropped (absmax impact 1e-9).
                g = outp.tile([128, S], f32)
                if r0 > 0:
                    nc.scalar.activation(
                        g[:, 0:r0], cumrow[:, 0:r0], AF.Exp,
                        bias=cumcols[:, t:t + 1], scale=-1.0,
                    )
                nc.scalar.activation(
                    g[:, r0 + 128:S], cumrow[:, r0 + 128:S], AF.Exp,
                    bias=ncc[:, t:t + 1], scale=1.0,
                )
                mid_r = gwin.tile([128, 128], f32)
                nc.scalar.activation(
                    mid_r, cumrow[:, r0:r0 + 128], AF.Exp,
                    bias=ncc[:, t:t + 1], scale=1.0,
                )
                mid_l = gwin.tile([128, 128], f32)
                nc.scalar.activation(
                    mid_l, cumrow[:, r0:r0 + 128], AF.Exp,
                    bias=cumcols[:, t:t + 1], scale=-1.0,
                )
                nc.vector.tensor_tensor(g[:, r0:r0 + 128], mid_r, mid_l, OP.min)
                nc.sync.dma_start(out=out_g[r0:r0 + 128, :], in_=g)

            # band diagonals straight into DRAM (strided DRAM->DRAM copies)
            def diag_ap(dt, offset, count):
                return bass.AP(tensor=dt[:, :].tensor, offset=dt[:, :].offset + offset,
                               ap=[[S + 1, count]])

            nc.sync.dma_start(out=diag_ap(out_nb, 1, HALF), in_=dsup_d[1:1 + HALF])
            nc.sync.dma_start(out=diag_ap(out_nb, S, HALF - 1),
                              in_=dsup_d[1:HALF])
            nc.sync.dma_start(out=diag_ap(out_nb, 0, HALF), in_=dmain_d[0:HALF])
            nc.sync.dma_start(out=diag_ap(out_g, 0, HALF), in_=dmain_d[0:HALF])

    nc.compile()
    return nc


def _consts():
    k = np.arange(128)
    lt = (k[:, None] < k[None, :]).astype(np.float32)       # lt[k,p]=k<p
    wup = (k[None, :] > k[:, None]).astype(np.float32)      # wup[p,w]=w>p
    wlo = (k[None, :] < k[:, None]).astype(np.float32)
    import ml_dtypes
    ones = np.ones((128, 1), dtype=ml_dtypes.bfloat16)
    zer = np.zeros(16, np.float32)
    return lt, wup, wlo, ones, zer


def kernel(context, eos_mask, prior, wq, bq, wk, bk, gamma, beta):
    from concourse.bass_utils import run_bass_kernel_spmd

    if "nc" not in _cache:
        _cache["nc"] = _build()
    nc = _cache["nc"]

    context = np.asarray(context, np.float32)
    eos_mask = np.asarray(eos_mask, np.int32)
    prior = np.asarray(prior, np.float32)
    wq = np.asarray(wq, np.float32)
    wk = np.asarray(wk, np.float32)
    lt, wup, wlo, ones, zer = _consts()

    in_maps = []
    for c in range(8):
        b, h = c // 2, c % 2
        x = context[b] if h == 0 else context[b][::-1]
        eo = eos_mask[b] if h == 0 else eos_mask[b][::-1]
        eop = np.zeros(S + 2, np.int32)
        eop[1:S + 1] = eo
        in_maps.append({
            "x": np.ascontiguousarray(x),
            "eospad": eop,
            "prior": prior,
            "wq": wq, "wk": wk,
            "lt128": lt, "wup": wup, "wlo": wlo,
            "onesb": ones, "zerosf": zer,
        })

    bkr = run_bass_kernel_spmd(nc, in_maps, core_ids=list(range(8)))
    _cache["last_bkr"] = bkr

    g_out = np.empty((B, S, S), np.float32)
    nb_out = np.empty((B, S, S), np.float32)
    for c in range(8):
        b, h = c // 2, c % 2
        rg = bkr.results[c]["out_g"]
        rn = bkr.results[c]["out_nb"]
        if h == 0:
            g_out[b, :HALF] = rg
            nb_out[b, :HALF] = rn
        else:
            g_out[b, HALF:] = rg[::-1, ::-1]
            nb_out[b, HALF:] = rn[::-1, ::-1]
    return g_out, nb_out



# revision 11
# speedup vs baseline: 1.9531x; 1.0252x over previous
"""GroupAttention sparse-attention kernel for 8 trn2 NeuronCores.

Math (derived + numerically verified against the reference):
  - The mask keeps only tridiagonal scores -> softmax rows have >=1 finite
    entries at j=i+-1, or are fully uniform 1/S ("caseB" rows, where
    eos[i-1]=eos[i+1]=0).
  - neibor = v0 + (vBB-v0)*u u^T  (rank-1 over caseB flags u), overwritten on
    the 3 band diagonals with d_sup/d_main.
  - g[i,j] = exp(cum[j]-cum[i]) for j>i (sym.), diag d_main, +1e-9 off-diag,
    where cum = prefix-sum of ell=log(d_sup+1e-9).
  - scores use A~ = wq^T wk:  s[i,j] = xn_i A~ xn_j^T / 512.
SPMD: one program "compute rows 0..1023". core 2b -> batch b as-is;
core 2b+1 -> batch b with rows reversed (problem is reversal-covariant),
host un-reverses its output half. bq/bk/beta are zeros and gamma ones per the
problem spec, so they are folded away.

Scheduling notes (v2):
  - neibor tiles depend only on eos_mask -> generated and DMA'd at t=0,
    fully overlapped with the matmul pipeline.
  - inputs are loaded with contiguous-per-partition (p t) layouts (32KB+
    descriptors); the contraction order over features is permuted
    accordingly (sums are order-invariant).
  - LN runs in 4 sequence quarters; transposes per half -> z matmuls start
    earlier.
  - two-entry softmax == sigmoid(+-(s_next - s_prev)/512).
  - both band reductions accumulate into one [64,S] PSUM tile (rows 0/32).
  - g rows are built on ACT directly: Exp(+-cumrow + bias) per known-sign
    segment, min(exp(d),exp(-d)) on the diagonal block.
"""

import numpy as np
from contextlib import ExitStack

B, S, D = 4, 2048, 1024
NT = 8          # 128-row output blocks per core (half of S/128)
HALF = S // 2

_cache = {}


def _build():
    import concourse.bass as bass
    import concourse.bacc as bacc
    import concourse.mybir as mybir
    from concourse.tile import TileContext

    f32 = mybir.dt.float32
    bf16 = mybir.dt.bfloat16
    i32 = mybir.dt.int32
    AF = mybir.ActivationFunctionType
    OP = mybir.AluOpType

    nc = bacc.Bacc("TRN2", target_bir_lowering=False)

    # ---------------- I/O ----------------
    x_in = nc.dram_tensor("x", [S, D], f32, kind="ExternalInput")
    eospad = nc.dram_tensor("eospad", [S + 2], i32, kind="ExternalInput")
    prior_t = nc.dram_tensor("prior", [1], f32, kind="ExternalInput")
    wq_in = nc.dram_tensor("wq", [D, D], f32, kind="ExternalInput")
    wk_in = nc.dram_tensor("wk", [D, D], f32, kind="ExternalInput")
    lt_in = nc.dram_tensor("lt128", [128, 128], f32, kind="ExternalInput")
    ones_in = nc.dram_tensor("onesb", [128, 1], bf16, kind="ExternalInput")
    out_nb = nc.dram_tensor("out_nb", [HALF, S], f32, kind="ExternalOutput")
    out_g = nc.dram_tensor("out_g", [HALF, S], f32, kind="ExternalOutput")

    C_SQ9 = float(np.sqrt(np.float32(1e-9)))                    # sqrt(1e-9)
    C_SBB = float(np.sqrt(np.float32((1.0 / S) ** 2 + 1e-9)))   # caseB diag sqrt

    with TileContext(nc) as tc, ExitStack() as ctx:
        # ---------------- pools (whole-kernel lifetime) ----------------
        consts = ctx.enter_context(tc.tile_pool(name="consts", bufs=1))
        vec = ctx.enter_context(tc.tile_pool(name="vec", bufs=28))
        col = ctx.enter_context(tc.tile_pool(name="col", bufs=10))
        at_pool = ctx.enter_context(tc.tile_pool(name="atp", bufs=1))
        xnt_pool = ctx.enter_context(tc.tile_pool(name="xntp", bufs=1))
        psA = ctx.enter_context(tc.tile_pool(name="psA", bufs=2, space="PSUM"))
        psB = ctx.enter_context(tc.tile_pool(name="psB", bufs=1, space="PSUM"))
        dram = ctx.enter_context(tc.tile_pool(name="dram", bufs=1, space="DRAM"))
        outp = ctx.enter_context(tc.tile_pool(name="outp", bufs=3))
        bcast = ctx.enter_context(tc.tile_pool(name="bcast", bufs=2))
        colp = ctx.enter_context(tc.tile_pool(name="colp", bufs=1))

        # ---------------- consts into SBUF ----------------
        lt128 = consts.tile([128, 128], f32)
        nc.sync.dma_start(out=lt128, in_=lt_in[:, :])
        ones_b = consts.tile([128, 1], bf16)
        nc.sync.dma_start(out=ones_b, in_=ones_in[:, :])
        pr_col = consts.tile([128, 1], f32)
        nc.sync.dma_start(
            out=pr_col,
            in_=bass.AP(tensor=prior_t[:].tensor, offset=prior_t[:].offset, ap=[[0, 128], [1, 1]]),
        )
        omp_col = consts.tile([128, 1], f32)  # 1 - prior
        nc.vector.tensor_scalar(omp_col, pr_col, -1.0, 1.0, OP.mult, OP.add)
        # v0 / vBB / (vBB-v0) as [128,1] broadcast columns
        v0_col = consts.tile([128, 1], f32)
        nc.vector.tensor_scalar(v0_col, omp_col, C_SQ9, None, OP.mult)
        nc.vector.tensor_tensor(v0_col, v0_col, pr_col, OP.add)
        vbb_col = consts.tile([128, 1], f32)
        nc.vector.tensor_scalar(vbb_col, omp_col, C_SBB, None, OP.mult)
        nc.vector.tensor_tensor(vbb_col, vbb_col, pr_col, OP.add)
        dv_col = consts.tile([128, 1], f32)  # vBB - v0
        nc.vector.tensor_tensor(dv_col, vbb_col, v0_col, OP.subtract)
        neg9 = consts.tile([128, 16], f32)
        nc.vector.memset(neg9, -1.0e9)
        # register const bias columns used by activation(bias=float)
        for ci, cval in enumerate((0.0, 1e-9, 1e-5)):
            cc = consts.tile([128, 1], f32, name=f"cc{ci}", tag=f"cc{ci}")
            nc.vector.memset(cc, cval)
            nc.const_aps.aps[(f32, cval)] = cc[:, :]

        # ---------------- DRAM scratch ----------------
        xb_d = dram.tile([S, D], bf16)          # normalized x, bf16
        snext_d = dram.tile([S], f32)
        sprev_d = dram.tile([S], f32)
        cum_d = dram.tile([S], f32)
        uscl_d = dram.tile([S], f32)            # (vBB-v0)*u
        u_d = dram.tile([S], f32)
        dsup_d = dram.tile([S + 1], f32)        # [0]=0, [1+i]=d_sup[i]
        dmain_d = dram.tile([S], f32)

        def v16(nm="v16"):
            return vec.tile([128, 16], f32, tag="v16", name=nm)

        def rd16(dtensor, off):  # dram vec [off:off+2048] -> [128,16] row-major
            return dtensor[off:off + S].rearrange("(p c) -> p c", c=16)

        def wr16(dtensor, off, src):
            nc.sync.dma_start(
                out=dtensor[off:off + S].rearrange("(p c) -> p c", c=16), in_=src
            )

        # ======== phase 0: caseB flags from eos only; neibor tiles ========
        hn_i = vec.tile([128, 16], i32)
        nc.sync.dma_start(out=hn_i, in_=rd16(eospad[:], 2))
        hp_i = vec.tile([128, 16], i32)
        nc.sync.dma_start(out=hp_i, in_=rd16(eospad[:], 0))
        hn = v16("hn")
        nc.vector.tensor_copy(out=hn, in_=hn_i)
        hp = v16("hp")
        nc.vector.tensor_copy(out=hp, in_=hp_i)
        # u = (1-hn)*(1-hp)
        t1 = v16("t1")
        nc.vector.tensor_scalar(t1, hn, -1.0, 1.0, OP.mult, OP.add)
        t2 = v16("t2")
        nc.vector.tensor_scalar(t2, hp, -1.0, 1.0, OP.mult, OP.add)
        cb = v16("cb")
        nc.vector.tensor_tensor(cb, t1, t2, OP.mult)
        omcb = v16("omcb")
        nc.vector.tensor_scalar(omcb, cb, -1.0, 1.0, OP.mult, OP.add)
        cbS = v16("cbS")
        nc.vector.tensor_scalar(cbS, cb, 1.0 / S, None, OP.mult)
        uscl = v16("uscl")
        nc.vector.tensor_scalar(uscl, cb, dv_col, None, OP.mult)
        wr16(uscl_d, 0, uscl)
        wr16(u_d, 0, cb)

        urow = bcast.tile([128, S], f32, tag="brow", name="urow")
        nc.sync.dma_start(
            out=urow,
            in_=bass.AP(tensor=uscl_d[:].tensor, offset=uscl_d[:].offset,
                        ap=[[0, 128], [1, S]]),
        )
        ucols = colp.tile([128, 8], f32)
        nc.sync.dma_start(
            out=ucols, in_=u_d[0:HALF].rearrange("(t p) -> p t", p=128)
        )
        for t in range(NT):
            r0 = t * 128
            nb = outp.tile([128, S], f32, tag="ot", name="nb")
            nc.vector.tensor_scalar(
                nb, urow, ucols[:, t:t + 1], v0_col, OP.mult, OP.add
            )
            nc.sync.dma_start(out=out_nb[r0:r0 + 128, :], in_=nb)

        # ============ phase 1: A~^T (bf16) ; LN+cast x ============
        with ExitStack() as p1:
            wpool = p1.enter_context(tc.tile_pool(name="wpool", bufs=1))
            xpool = p1.enter_context(tc.tile_pool(name="xpool", bufs=2))
            xbpool = p1.enter_context(tc.tile_pool(name="xbpool", bufs=2))
            stpool = p1.enter_context(tc.tile_pool(name="stpool", bufs=8))

            # weights: contiguous 32KB/partition loads; wqb[p,t,e]=wq[p*8+t,e]
            # (the A~ contraction below enumerates f=p*8+dt -- order-invariant)
            wqb = wpool.tile([128, 8, D], bf16)
            nc.gpsimd.dma_start(
                out=wqb[:, :, :], in_=wq_in[:, :].rearrange("(p t) e -> p t e", p=128)
            )
            wkb = wpool.tile([128, 8, D], bf16)
            nc.gpsimd.dma_start(
                out=wkb[:, :, :], in_=wk_in[:, :].rearrange("(p t) e -> p t e", p=128)
            )

            at_sb = at_pool.tile([128, 8, D], bf16)  # AT[p,ft,e] = A~^T[f,e]
            for ft in range(8):
                ps = psA.tile([128, D], f32)
                for dt in range(8):
                    for c in range(2):
                        nc.tensor.matmul(
                            ps[:, c * 512:(c + 1) * 512],
                            wkb[:, dt, ft * 128:(ft + 1) * 128],
                            wqb[:, dt, c * 512:(c + 1) * 512],
                            start=(dt == 0),
                            stop=(dt == 7),
                        )
                if ft % 2 == 0:
                    nc.vector.tensor_copy(out=at_sb[:, ft, :], in_=ps[:, :])
                else:
                    nc.scalar.copy(out=at_sb[:, ft, :], in_=ps[:, :])

            # --- LN in 4 sequence quarters; rows (p t): i = q*512 + p*4 + t ---
            for q in range(4):
                xq = xpool.tile([128, 4, D], f32)
                nc.sync.dma_start(
                    out=xq,
                    in_=x_in[q * 512:(q + 1) * 512, :].rearrange(
                        "(p t) e -> p t e", p=128),
                )
                xbq = xbpool.tile([128, 4, D], bf16)
                for t in range(4):
                    stats = stpool.tile([128, 2, 6], f32)
                    nc.vector.bn_stats(out=stats[:, 0, :], in_=xq[:, t, 0:512])
                    nc.vector.bn_stats(out=stats[:, 1, :], in_=xq[:, t, 512:1024])
                    mv = stpool.tile([128, 2], f32)
                    nc.vector.bn_aggr(out=mv, in_=stats)
                    # rstd = 1/sqrt(var+1e-5); Sqrt keeps one ACT set resident
                    sdt = stpool.tile([128, 1], f32)
                    nc.scalar.activation(sdt, mv[:, 1:2], AF.Sqrt, bias=1e-5)
                    rstd = stpool.tile([128, 1], f32)
                    nc.vector.reciprocal(rstd, sdt)
                    nc.vector.tensor_scalar(
                        xbq[:, t, :], xq[:, t, :], mv[:, 0:1], rstd,
                        OP.subtract, OP.mult
                    )
                nc.sync.dma_start(
                    out=xb_d[q * 512:(q + 1) * 512, :].rearrange(
                        "(p t) e -> p t e", p=128),
                    in_=xbq,
                )

        # ============ phase 2: transpose; z; band dot-products ============
        xnt = xnt_pool.tile([128, 8, S], bf16)   # xnt[p,ft,i] = xn[i, ft*128+p]
        for h in range(2):
            for ft in range(8):
                nc.sync.dma_start(
                    out=xnt[:, ft, h * 1024:(h + 1) * 1024],
                    in_=xb_d[h * 1024:(h + 1) * 1024, ft * 128:(ft + 1) * 128],
                    transpose=True,
                )

        with ExitStack() as p2:
            zpool = p2.enter_context(tc.tile_pool(name="zpool", bufs=2))
            ppool = p2.enter_context(tc.tile_pool(name="ppool", bufs=4))
            rows = p2.enter_context(tc.tile_pool(name="rows", bufs=2))

            # both band-dot reductions live in one PSUM tile: row 0 = s_next,
            # row 32 = s_prev (32-aligned partition groups)
            psrow = psB.tile([64, S], f32, tag="psrow", name="psrow")
            for et in range(8):
                zb = zpool.tile([128, S], bf16)
                for half in range(2):
                    ps = psA.tile([128, 1024], f32)
                    for ft in range(8):
                        for c in range(2):
                            off = half * 1024 + c * 512
                            nc.tensor.matmul(
                                ps[:, c * 512:(c + 1) * 512],
                                at_sb[:, ft, et * 128:(et + 1) * 128],
                                xnt[:, ft, off:off + 512],
                                start=(ft == 0),
                                stop=(ft == 7),
                            )
                    nc.scalar.copy(out=zb[:, half * 1024:(half + 1) * 1024], in_=ps)
                # band products
                pt1 = ppool.tile([128, S], bf16, tag="pt1", name="pt1")
                nc.vector.tensor_tensor(
                    pt1[:, 0:S - 1], xnt[:, et, 0:S - 1], zb[:, 1:S], OP.mult
                )
                pt2 = ppool.tile([128, S], bf16, tag="pt2", name="pt2")
                nc.vector.tensor_tensor(
                    pt2[:, 1:S], xnt[:, et, 1:S], zb[:, 0:S - 1], OP.mult
                )
                for c in range(4):
                    nc.tensor.matmul(
                        psrow[0:1, c * 512:(c + 1) * 512],
                        ones_b,
                        pt1[:, c * 512:(c + 1) * 512],
                        start=(et == 0),
                        stop=(et == 7),
                    )
                    nc.tensor.matmul(
                        psrow[32:33, c * 512:(c + 1) * 512],
                        ones_b,
                        pt2[:, c * 512:(c + 1) * 512],
                        start=(et == 0),
                        stop=(et == 7),
                    )
            row_n = rows.tile([1, S], f32)
            nc.scalar.copy(out=row_n, in_=psrow[0:1, :])
            nc.sync.dma_start(out=snext_d[:], in_=row_n)
            row_p = rows.tile([1, S], f32)
            nc.scalar.copy(out=row_p, in_=psrow[32:33, :])
            nc.sync.dma_start(out=sprev_d[:], in_=row_p)

        # ============ phase 3: band math in [128,16] layout ============
        sn = v16("sn")
        nc.sync.dma_start(out=sn, in_=rd16(snext_d, 0))
        sp = v16("sp")
        nc.sync.dma_start(out=sp, in_=rd16(sprev_d, 0))

        sne = v16("sne")
        nc.vector.select(sne, hn_i, sn, neg9)
        spe = v16("spe")
        nc.vector.select(spe, hp_i, sp, neg9)
        # two-entry softmax == sigmoid; raw scores are scaled by 1/512 here
        dd = v16("dd")
        nc.vector.tensor_tensor(dd, sne, spe, OP.subtract)
        nn = v16("nn")
        nc.scalar.activation(nn, dd, AF.Sigmoid, scale=1.0 / 512.0)
        npv = v16("npv")
        nc.scalar.activation(npv, dd, AF.Sigmoid, scale=-1.0 / 512.0)
        # blend caseB rows to uniform 1/S
        for nv in (nn, npv):
            nc.vector.tensor_tensor(nv, nv, omcb, OP.mult)
            nc.vector.tensor_tensor(nv, nv, cbS, OP.add)
        # Np shifted by +1 (value at i+1)
        npsh = v16("npsh")
        nc.vector.memset(npsh, 0.0)
        nc.vector.tensor_copy(out=npsh[:, 0:15], in_=npv[:, 1:16])
        nc.sync.dma_start(out=npsh[0:127, 15:16], in_=npv[1:128, 0:1])
        msup = v16("msup")
        nc.vector.tensor_tensor(msup, nn, npsh, OP.mult)
        # d_sup = prior + (1-prior)*sqrt(msup+1e-9)
        dsup = v16("dsup")
        nc.scalar.activation(dsup, msup, AF.Sqrt, bias=1e-9)
        nc.vector.tensor_scalar(dsup, dsup, omp_col, pr_col, OP.mult, OP.add)
        # d_main = prior + (1-prior)*(c1 + (c2-c1)*cb)
        dmain = v16("dmain")
        nc.vector.tensor_scalar(dmain, cb, C_SBB - C_SQ9, C_SQ9, OP.mult, OP.add)
        nc.vector.tensor_scalar(dmain, dmain, omp_col, pr_col, OP.mult, OP.add)
        # ell, prefix sums
        ell = v16("ell")
        nc.scalar.activation(ell, dsup, AF.Ln, bias=1e-9)
        zv16 = v16("zv16")
        nc.vector.memset(zv16, 0.0)
        incl = v16("incl")
        nc.vector.tensor_tensor_scan(incl, ell, zv16, 0.0, OP.add, OP.add)
        excl = v16("excl")
        nc.vector.tensor_tensor(excl, incl, ell, OP.subtract)
        ps_c = psA.tile([128, 1024], f32, tag="ps", name="ps_c")
        nc.tensor.matmul(
            ps_c[:, 0:1], lt128, incl[:, 15:16], start=True, stop=True
        )
        cp_col = col.tile([128, 1], f32)
        nc.vector.tensor_copy(out=cp_col, in_=ps_c[:, 0:1])
        cum = v16("cum")
        nc.vector.tensor_scalar(cum, excl, cp_col, None, OP.add)

        wr16(cum_d, 0, cum)
        wr16(dsup_d, 1, dsup)
        wr16(dmain_d, 0, dmain)

        # ============ phase 4: g output ============
        cumrow = bcast.tile([128, S], f32, tag="brow", name="cumrow")
        nc.sync.dma_start(
            out=cumrow,
            in_=bass.AP(tensor=cum_d[:].tensor, offset=cum_d[:].offset,
                        ap=[[0, 128], [1, S]]),
        )
        cumcols = colp.tile([128, 8], f32)
        nc.sync.dma_start(
            out=cumcols, in_=cum_d[0:HALF].rearrange("(t p) -> p t", p=128)
        )
        ncc = colp.tile([128, 8], f32)  # -cum_i (bias for right/upper exp)
        nc.vector.tensor_scalar(ncc, cumcols, -1.0, None, OP.mult)

        with ExitStack() as p3:
            gwin = p3.enter_context(tc.tile_pool(name="gwin", bufs=4))

            for t in range(NT):
                r0 = t * 128
                # g = exp(-|cum_j - cum_i|), built on ACT: per-segment the sign
                # of (cum_j - cum_i) is known, so Exp(scale*cumrow + bias) is
                # safe; the 128-wide diagonal block uses min(exp(d), exp(-d)).
                # Diagonal patched later via diag DMA; reference's +1e-9 is
                # dropped (absmax impact 1e-9).
                g = outp.tile([128, S], f32, tag="ot", name="g")
                if r0 > 0:
                    nc.scalar.activation(
                        g[:, 0:r0], cumrow[:, 0:r0], AF.Exp,
                        bias=cumcols[:, t:t + 1], scale=-1.0,
                    )
                nc.scalar.activation(
                    g[:, r0 + 128:S], cumrow[:, r0 + 128:S], AF.Exp,
                    bias=ncc[:, t:t + 1], scale=1.0,
                )
                mid_r = gwin.tile([128, 128], f32)
                nc.scalar.activation(
                    mid_r, cumrow[:, r0:r0 + 128], AF.Exp,
                    bias=ncc[:, t:t + 1], scale=1.0,
                )
                mid_l = gwin.tile([128, 128], f32)
                nc.scalar.activation(
                    mid_l, cumrow[:, r0:r0 + 128], AF.Exp,
                    bias=cumcols[:, t:t + 1], scale=-1.0,
                )
                nc.vector.tensor_tensor(g[:, r0:r0 + 128], mid_r, mid_l, OP.min)
                nc.sync.dma_start(out=out_g[r0:r0 + 128, :], in_=g)

            # band diagonals straight into DRAM (strided DRAM->DRAM copies)
            def diag_ap(dt, offset, count):
                return bass.AP(tensor=dt[:, :].tensor, offset=dt[:, :].offset + offset,
                               ap=[[S + 1, count]])

            nc.sync.dma_start(out=diag_ap(out_nb, 1, HALF), in_=dsup_d[1:1 + HALF])
            nc.sync.dma_start(out=diag_ap(out_nb, S, HALF - 1),
                              in_=dsup_d[1:HALF])
            nc.sync.dma_start(out=diag_ap(out_nb, 0, HALF), in_=dmain_d[0:HALF])
            nc.sync.dma_start(out=diag_ap(out_g, 0, HALF), in_=dmain_d[0:HALF])

    nc.compile()
    return nc


def _consts():
    k = np.arange(128)
    lt = (k[:, None] < k[None, :]).astype(np.float32)       # lt[k,p]=k<p
    import ml_dtypes
    ones = np.ones((128, 1), dtype=ml_dtypes.bfloat16)
    return lt, ones


def kernel(context, eos_mask, prior, wq, bq, wk, bk, gamma, beta):
    from concourse.bass_utils import run_bass_kernel_spmd

    if "nc" not in _cache:
        _cache["nc"] = _build()
    nc = _cache["nc"]

    context = np.asarray(context, np.float32)
    eos_mask = np.asarray(eos_mask, np.int32)
    prior = np.asarray(prior, np.float32)
    wq = np.asarray(wq, np.float32)
    wk = np.asarray(wk, np.float32)
    lt, ones = _consts()

    in_maps = []
    for c in range(8):
        b, h = c // 2, c % 2
        x = context[b] if h == 0 else context[b][::-1]
        eo = eos_mask[b] if h == 0 else eos_mask[b][::-1]
        eop = np.zeros(S + 2, np.int32)
        eop[1:S + 1] = eo
        in_maps.append({
            "x": np.ascontiguousarray(x),
            "eospad": eop,
            "prior": prior,
            "wq": wq, "wk": wk,
            "lt128": lt,
            "onesb": ones,
        })

    bkr = run_bass_kernel_spmd(nc, in_maps, core_ids=list(range(8)))
    _cache["last_bkr"] = bkr

    g_out = np.empty((B, S, S), np.float32)
    nb_out = np.empty((B, S, S), np.float32)
    for c in range(8):
        b, h = c // 2, c % 2
        rg = bkr.results[c]["out_g"]
        rn = bkr.results[c]["out_nb"]
        if h == 0:
            g_out[b, :HALF] = rg
            nb_out[b, :HALF] = rn
        else:
            g_out[b, HALF:] = rg[::-1, ::-1]
            nb_out[b, HALF:] = rn[::-1, ::-1]
    return g_out, nb_out


# revision 19
# speedup vs baseline: 2.0588x; 1.0541x over previous
"""GroupAttention sparse-attention kernel for 8 trn2 NeuronCores.

Math (derived + numerically verified against the reference):
  - The mask keeps only tridiagonal scores -> softmax rows have >=1 finite
    entries at j=i+-1, or are fully uniform 1/S ("caseB" rows, where
    eos[i-1]=eos[i+1]=0).
  - neibor = v0 + (vBB-v0)*u u^T  (rank-1 over caseB flags u), overwritten on
    the 3 band diagonals with d_sup/d_main.
  - g[i,j] = exp(cum[j]-cum[i]) for j>i (sym.), diag d_main, +1e-9 off-diag,
    where cum = prefix-sum of ell=log(d_sup+1e-9).
  - scores use A~ = wq^T wk:  s[i,j] = xn_i A~ xn_j^T / 512.
SPMD: one program "compute rows 0..1023". core 2b -> batch b as-is;
core 2b+1 -> batch b with rows reversed (problem is reversal-covariant),
host un-reverses its output half. bq/bk/beta are zeros and gamma ones per the
problem spec, so they are folded away.

Scheduling notes (v2):
  - neibor tiles depend only on eos_mask -> generated and DMA'd at t=0,
    fully overlapped with the matmul pipeline.
  - inputs are loaded with contiguous-per-partition (p t) layouts (32KB+
    descriptors); the contraction order over features is permuted
    accordingly (sums are order-invariant).
  - LN runs in 4 sequence quarters; transposes per half -> z matmuls start
    earlier.
  - two-entry softmax == sigmoid(+-(s_next - s_prev)/512).
  - both band reductions accumulate into one [64,S] PSUM tile (rows 0/32).
  - g rows are built on ACT directly: Exp(+-cumrow + bias) per known-sign
    segment, min(exp(d),exp(-d)) on the diagonal block.
"""

import numpy as np
from contextlib import ExitStack

B, S, D = 4, 2048, 1024
NT = 8          # 128-row output blocks per core (half of S/128)
HALF = S // 2

_cache = {}


def _build():
    import concourse.bass as bass
    import concourse.bacc as bacc
    import concourse.mybir as mybir
    from concourse.tile import TileContext

    f32 = mybir.dt.float32
    bf16 = mybir.dt.bfloat16
    i32 = mybir.dt.int32
    AF = mybir.ActivationFunctionType
    OP = mybir.AluOpType

    nc = bacc.Bacc("TRN2", target_bir_lowering=False)

    # ---------------- I/O ----------------
    # x/wq/wk arrive as bf16 (host-cast): halves input HBM traffic; the
    # pipeline computes in bf16 anyway.
    x_in = nc.dram_tensor("x", [S, D], bf16, kind="ExternalInput")
    eospad = nc.dram_tensor("eospad", [S + 2], i32, kind="ExternalInput")
    prior_t = nc.dram_tensor("prior", [1], f32, kind="ExternalInput")
    wq_in = nc.dram_tensor("wq", [D, D], bf16, kind="ExternalInput")
    wk_in = nc.dram_tensor("wk", [D, D], bf16, kind="ExternalInput")
    lt_in = nc.dram_tensor("lt128", [128, 128], f32, kind="ExternalInput")
    ones_in = nc.dram_tensor("onesb", [128, 1], bf16, kind="ExternalInput")
    out_nb = nc.dram_tensor("out_nb", [HALF, S], f32, kind="ExternalOutput")
    out_g = nc.dram_tensor("out_g", [HALF, S], f32, kind="ExternalOutput")

    C_SQ9 = float(np.sqrt(np.float32(1e-9)))                    # sqrt(1e-9)
    C_SBB = float(np.sqrt(np.float32((1.0 / S) ** 2 + 1e-9)))   # caseB diag sqrt

    with TileContext(nc) as tc, ExitStack() as ctx:
        # ---------------- pools (whole-kernel lifetime) ----------------
        consts = ctx.enter_context(tc.tile_pool(name="consts", bufs=1))
        vec = ctx.enter_context(tc.tile_pool(name="vec", bufs=28))
        col = ctx.enter_context(tc.tile_pool(name="col", bufs=10))
        at_pool = ctx.enter_context(tc.tile_pool(name="atp", bufs=1))
        xnt_pool = ctx.enter_context(tc.tile_pool(name="xntp", bufs=1))
        psA = ctx.enter_context(tc.tile_pool(name="psA", bufs=2, space="PSUM"))
        psB = ctx.enter_context(tc.tile_pool(name="psB", bufs=1, space="PSUM"))
        dram = ctx.enter_context(tc.tile_pool(name="dram", bufs=1, space="DRAM"))
        outp = ctx.enter_context(tc.tile_pool(name="outp", bufs=3))
        bcast = ctx.enter_context(tc.tile_pool(name="bcast", bufs=2))
        colp = ctx.enter_context(tc.tile_pool(name="colp", bufs=1))

        # ---------------- consts into SBUF ----------------
        lt128 = consts.tile([128, 128], f32)
        nc.sync.dma_start(out=lt128, in_=lt_in[:, :])
        ones_b = consts.tile([128, 1], bf16)
        nc.sync.dma_start(out=ones_b, in_=ones_in[:, :])
        pr_col = consts.tile([128, 1], f32)
        nc.sync.dma_start(
            out=pr_col,
            in_=bass.AP(tensor=prior_t[:].tensor, offset=prior_t[:].offset, ap=[[0, 128], [1, 1]]),
        )
        omp_col = consts.tile([128, 1], f32)  # 1 - prior
        nc.vector.tensor_scalar(omp_col, pr_col, -1.0, 1.0, OP.mult, OP.add)
        # v0 / vBB / (vBB-v0) as [128,1] broadcast columns
        v0_col = consts.tile([128, 1], f32)
        nc.vector.tensor_scalar(v0_col, omp_col, C_SQ9, None, OP.mult)
        nc.vector.tensor_tensor(v0_col, v0_col, pr_col, OP.add)
        vbb_col = consts.tile([128, 1], f32)
        nc.vector.tensor_scalar(vbb_col, omp_col, C_SBB, None, OP.mult)
        nc.vector.tensor_tensor(vbb_col, vbb_col, pr_col, OP.add)
        dv_col = consts.tile([128, 1], f32)  # vBB - v0
        nc.vector.tensor_tensor(dv_col, vbb_col, v0_col, OP.subtract)
        neg9 = consts.tile([128, 16], f32)
        nc.vector.memset(neg9, -1.0e9)
        # register const bias columns used by activation(bias=float)
        for ci, cval in enumerate((0.0, 1e-9, 1e-5)):
            cc = consts.tile([128, 1], f32, name=f"cc{ci}", tag=f"cc{ci}")
            nc.vector.memset(cc, cval)
            nc.const_aps.aps[(f32, cval)] = cc[:, :]

        # ---------------- DRAM scratch ----------------
        xb_d = dram.tile([S, D], bf16)          # normalized x, bf16
        snext_d = dram.tile([S], f32)
        sprev_d = dram.tile([S], f32)
        cum_d = dram.tile([S], f32)
        uscl_d = dram.tile([S], f32)            # (vBB-v0)*u
        u_d = dram.tile([S], f32)
        dsup_d = dram.tile([S + 1], f32)        # [0]=0, [1+i]=d_sup[i]
        dmain_d = dram.tile([S], f32)

        def v16(nm="v16"):
            return vec.tile([128, 16], f32, tag="v16", name=nm)

        def rd16(dtensor, off):  # dram vec [off:off+2048] -> [128,16] row-major
            return dtensor[off:off + S].rearrange("(p c) -> p c", c=16)

        def wr16(dtensor, off, src):
            nc.sync.dma_start(
                out=dtensor[off:off + S].rearrange("(p c) -> p c", c=16), in_=src
            )

        # ======== phase 0: caseB flags from eos only; neibor tiles ========
        hn_i = vec.tile([128, 16], i32)
        nc.sync.dma_start(out=hn_i, in_=rd16(eospad[:], 2))
        hp_i = vec.tile([128, 16], i32)
        nc.sync.dma_start(out=hp_i, in_=rd16(eospad[:], 0))
        hn = v16("hn")
        nc.vector.tensor_copy(out=hn, in_=hn_i)
        hp = v16("hp")
        nc.vector.tensor_copy(out=hp, in_=hp_i)
        # u = (1-hn)*(1-hp)
        t1 = v16("t1")
        nc.vector.tensor_scalar(t1, hn, -1.0, 1.0, OP.mult, OP.add)
        t2 = v16("t2")
        nc.vector.tensor_scalar(t2, hp, -1.0, 1.0, OP.mult, OP.add)
        cb = v16("cb")
        nc.vector.tensor_tensor(cb, t1, t2, OP.mult)
        omcb = v16("omcb")
        nc.vector.tensor_scalar(omcb, cb, -1.0, 1.0, OP.mult, OP.add)
        cbS = v16("cbS")
        nc.vector.tensor_scalar(cbS, cb, 1.0 / S, None, OP.mult)
        uscl = v16("uscl")
        nc.vector.tensor_scalar(uscl, cb, dv_col, None, OP.mult)
        wr16(uscl_d, 0, uscl)
        wr16(u_d, 0, cb)

        urow = bcast.tile([128, S], f32, tag="brow", name="urow")
        nc.sync.dma_start(
            out=urow,
            in_=bass.AP(tensor=uscl_d[:].tensor, offset=uscl_d[:].offset,
                        ap=[[0, 128], [1, S]]),
        )
        ucols = colp.tile([128, 8], f32)
        nc.sync.dma_start(
            out=ucols, in_=u_d[0:HALF].rearrange("(t p) -> p t", p=128)
        )
        for t in range(NT):
            r0 = t * 128
            nb = outp.tile([128, S], f32, tag="ot", name="nb")
            # on ACT (Identity is in every table set); keeps DVE free for LN
            nc.scalar.activation(
                nb, urow, AF.Identity, bias=v0_col, scale=ucols[:, t:t + 1]
            )
            nc.sync.dma_start(out=out_nb[r0:r0 + 128, :], in_=nb)

        # ============ phase 1: A~^T (bf16) ; LN+cast x ============
        with ExitStack() as p1:
            wpool = p1.enter_context(tc.tile_pool(name="wpool", bufs=1))
            xpool = p1.enter_context(tc.tile_pool(name="xpool", bufs=2))
            xbpool = p1.enter_context(tc.tile_pool(name="xbpool", bufs=2))
            stpool = p1.enter_context(tc.tile_pool(name="stpool", bufs=8))

            # weights: contiguous 16KB/partition loads; wqb[p,t,e]=wq[p*8+t,e]
            # (the A~ contraction below enumerates f=p*8+dt -- order-invariant)
            wqb = wpool.tile([128, 8, D], bf16)
            nc.sync.dma_start(
                out=wqb[:, :, :], in_=wq_in[:, :].rearrange("(p t) e -> p t e", p=128)
            )
            wkb = wpool.tile([128, 8, D], bf16)
            nc.scalar.dma_start(
                out=wkb[:, :, :], in_=wk_in[:, :].rearrange("(p t) e -> p t e", p=128)
            )

            at_sb = at_pool.tile([128, 8, D], bf16)  # AT[p,ft,e] = A~^T[f,e]
            for ft in range(8):
                ps = psA.tile([128, D], f32)
                for dt in range(8):
                    for c in range(2):
                        nc.tensor.matmul(
                            ps[:, c * 512:(c + 1) * 512],
                            wkb[:, dt, ft * 128:(ft + 1) * 128],
                            wqb[:, dt, c * 512:(c + 1) * 512],
                            start=(dt == 0),
                            stop=(dt == 7),
                        )
                nc.scalar.copy(out=at_sb[:, ft, :], in_=ps[:, :])

            # --- LN in 4 sequence quarters; rows (p t): i = q*512 + p*4 + t ---
            for q in range(4):
                xq = xpool.tile([128, 4, D], bf16)
                nc.sync.dma_start(
                    out=xq,
                    in_=x_in[q * 512:(q + 1) * 512, :].rearrange(
                        "(p t) e -> p t e", p=128),
                )
                xbq = xbpool.tile([128, 4, D], bf16)
                for t in range(4):
                    stats = stpool.tile([128, 2, 6], f32)
                    nc.vector.bn_stats(out=stats[:, 0, :], in_=xq[:, t, 0:512])
                    nc.vector.bn_stats(out=stats[:, 1, :], in_=xq[:, t, 512:1024])
                    mv = stpool.tile([128, 2], f32)
                    nc.vector.bn_aggr(out=mv, in_=stats)
                    # rstd = 1/sqrt(var+1e-5); Sqrt keeps one ACT set resident
                    sdt = stpool.tile([128, 1], f32)
                    nc.scalar.activation(sdt, mv[:, 1:2], AF.Sqrt, bias=1e-5)
                    rstd = stpool.tile([128, 1], f32)
                    nc.vector.reciprocal(rstd, sdt)
                    nc.vector.tensor_scalar(
                        xbq[:, t, :], xq[:, t, :], mv[:, 0:1], rstd,
                        OP.subtract, OP.mult
                    )
                nc.sync.dma_start(
                    out=xb_d[q * 512:(q + 1) * 512, :].rearrange(
                        "(p t) e -> p t e", p=128),
                    in_=xbq,
                )

        # ============ phase 2: transpose; z; band dot-products ============
        xnt = xnt_pool.tile([128, 8, S], bf16)   # xnt[p,ft,i] = xn[i, ft*128+p]
        for h in range(2):
            for ft in range(8):
                # all transposes on the ACT HWDGE ring: keeps the xbar on one
                # ring (two rings concurrently corrupt it) while the sync ring
                # stays free for bulk loads/stores
                nc.scalar.dma_start(
                    out=xnt[:, ft, h * 1024:(h + 1) * 1024],
                    in_=xb_d[h * 1024:(h + 1) * 1024, ft * 128:(ft + 1) * 128],
                    transpose=True,
                )

        with ExitStack() as p2:
            zpool = p2.enter_context(tc.tile_pool(name="zpool", bufs=2))
            ppool = p2.enter_context(tc.tile_pool(name="ppool", bufs=4))
            rows = p2.enter_context(tc.tile_pool(name="rows", bufs=2))

            # both band-dot reductions live in one PSUM tile: row 0 = s_next,
            # row 32 = s_prev (32-aligned partition groups)
            psrow = psB.tile([64, S], f32, tag="psrow", name="psrow")
            for et in range(8):
                zb = zpool.tile([128, S], bf16)
                for half in range(2):
                    ps = psA.tile([128, 1024], f32)
                    for ft in range(8):
                        for c in range(2):
                            off = half * 1024 + c * 512
                            nc.tensor.matmul(
                                ps[:, c * 512:(c + 1) * 512],
                                at_sb[:, ft, et * 128:(et + 1) * 128],
                                xnt[:, ft, off:off + 512],
                                start=(ft == 0),
                                stop=(ft == 7),
                            )
                    nc.scalar.copy(out=zb[:, half * 1024:(half + 1) * 1024], in_=ps)
                # band products
                pt1 = ppool.tile([128, S], bf16, tag="pt1", name="pt1")
                nc.vector.tensor_tensor(
                    pt1[:, 0:S - 1], xnt[:, et, 0:S - 1], zb[:, 1:S], OP.mult
                )
                pt2 = ppool.tile([128, S], bf16, tag="pt2", name="pt2")
                nc.vector.tensor_tensor(
                    pt2[:, 1:S], xnt[:, et, 1:S], zb[:, 0:S - 1], OP.mult
                )
                for c in range(4):
                    nc.tensor.matmul(
                        psrow[0:1, c * 512:(c + 1) * 512],
                        ones_b,
                        pt1[:, c * 512:(c + 1) * 512],
                        start=(et == 0),
                        stop=(et == 7),
                    )
                    nc.tensor.matmul(
                        psrow[32:33, c * 512:(c + 1) * 512],
                        ones_b,
                        pt2[:, c * 512:(c + 1) * 512],
                        start=(et == 0),
                        stop=(et == 7),
                    )
            row_n = rows.tile([1, S], f32)
            nc.scalar.copy(out=row_n, in_=psrow[0:1, :])
            nc.sync.dma_start(out=snext_d[:], in_=row_n)
            row_p = rows.tile([1, S], f32)
            nc.scalar.copy(out=row_p, in_=psrow[32:33, :])
            nc.sync.dma_start(out=sprev_d[:], in_=row_p)

        # ============ phase 3: band math in [128,16] layout ============
        sn = v16("sn")
        nc.sync.dma_start(out=sn, in_=rd16(snext_d, 0))
        sp = v16("sp")
        nc.sync.dma_start(out=sp, in_=rd16(sprev_d, 0))

        sne = v16("sne")
        nc.vector.select(sne, hn_i, sn, neg9)
        spe = v16("spe")
        nc.vector.select(spe, hp_i, sp, neg9)
        # two-entry softmax == sigmoid; raw scores are scaled by 1/512 here
        dd = v16("dd")
        nc.vector.tensor_tensor(dd, sne, spe, OP.subtract)
        nn = v16("nn")
        nc.scalar.activation(nn, dd, AF.Sigmoid, scale=1.0 / 512.0)
        npv = v16("npv")
        nc.scalar.activation(npv, dd, AF.Sigmoid, scale=-1.0 / 512.0)
        # blend caseB rows to uniform 1/S
        for nv in (nn, npv):
            nc.vector.tensor_tensor(nv, nv, omcb, OP.mult)
            nc.vector.tensor_tensor(nv, nv, cbS, OP.add)
        # Np shifted by +1 (value at i+1)
        npsh = v16("npsh")
        nc.vector.memset(npsh, 0.0)
        nc.vector.tensor_copy(out=npsh[:, 0:15], in_=npv[:, 1:16])
        nc.sync.dma_start(out=npsh[0:127, 15:16], in_=npv[1:128, 0:1])
        msup = v16("msup")
        nc.vector.tensor_tensor(msup, nn, npsh, OP.mult)
        # d_sup = prior + (1-prior)*sqrt(msup+1e-9)
        dsup = v16("dsup")
        nc.scalar.activation(dsup, msup, AF.Sqrt, bias=1e-9)
        nc.vector.tensor_scalar(dsup, dsup, omp_col, pr_col, OP.mult, OP.add)
        # d_main = prior + (1-prior)*(c1 + (c2-c1)*cb)
        dmain = v16("dmain")
        nc.vector.tensor_scalar(dmain, cb, C_SBB - C_SQ9, C_SQ9, OP.mult, OP.add)
        nc.vector.tensor_scalar(dmain, dmain, omp_col, pr_col, OP.mult, OP.add)
        # ell, prefix sums
        ell = v16("ell")
        nc.scalar.activation(ell, dsup, AF.Ln, bias=1e-9)
        zv16 = v16("zv16")
        nc.vector.memset(zv16, 0.0)
        incl = v16("incl")
        nc.vector.tensor_tensor_scan(incl, ell, zv16, 0.0, OP.add, OP.add)
        excl = v16("excl")
        nc.vector.tensor_tensor(excl, incl, ell, OP.subtract)
        ps_c = psA.tile([128, 1024], f32, tag="ps", name="ps_c")
        nc.tensor.matmul(
            ps_c[:, 0:1], lt128, incl[:, 15:16], start=True, stop=True
        )
        cp_col = col.tile([128, 1], f32)
        nc.vector.tensor_copy(out=cp_col, in_=ps_c[:, 0:1])
        cum = v16("cum")
        nc.vector.tensor_scalar(cum, excl, cp_col, None, OP.add)

        wr16(cum_d, 0, cum)
        wr16(dsup_d, 1, dsup)
        wr16(dmain_d, 0, dmain)

        # ============ phase 4: g output ============
        cumrow = bcast.tile([128, S], f32, tag="brow", name="cumrow")
        nc.sync.dma_start(
            out=cumrow,
            in_=bass.AP(tensor=cum_d[:].tensor, offset=cum_d[:].offset,
                        ap=[[0, 128], [1, S]]),
        )
        cumcols = colp.tile([128, 8], f32)
        nc.sync.dma_start(
            out=cumcols, in_=cum_d[0:HALF].rearrange("(t p) -> p t", p=128)
        )
        ncc = colp.tile([128, 8], f32)  # -cum_i (bias for right/upper exp)
        nc.vector.tensor_scalar(ncc, cumcols, -1.0, None, OP.mult)

        with ExitStack() as p3:
            gwin = p3.enter_context(tc.tile_pool(name="gwin", bufs=4))

            for t in range(NT):
                r0 = t * 128
                # g = exp(-|cum_j - cum_i|), built on ACT: per-segment the sign
                # of (cum_j - cum_i) is known, so Exp(scale*cumrow + bias) is
                # safe; the 128-wide diagonal block uses min(exp(d), exp(-d)).
                # Diagonal patched later via diag DMA; reference's +1e-9 is
                # dropped (absmax impact 1e-9).
                g = outp.tile([128, S], f32, tag="ot", name="g")
                if r0 > 0:
                    nc.scalar.activation(
                        g[:, 0:r0], cumrow[:, 0:r0], AF.Exp,
                        bias=cumcols[:, t:t + 1], scale=-1.0,
                    )
                nc.scalar.activation(
                    g[:, r0 + 128:S], cumrow[:, r0 + 128:S], AF.Exp,
                    bias=ncc[:, t:t + 1], scale=1.0,
                )
                mid_r = gwin.tile([128, 128], f32)
                nc.scalar.activation(
                    mid_r, cumrow[:, r0:r0 + 128], AF.Exp,
                    bias=ncc[:, t:t + 1], scale=1.0,
                )
                mid_l = gwin.tile([128, 128], f32)
                nc.scalar.activation(
                    mid_l, cumrow[:, r0:r0 + 128], AF.Exp,
                    bias=cumcols[:, t:t + 1], scale=-1.0,
                )
                nc.vector.tensor_tensor(g[:, r0:r0 + 128], mid_r, mid_l, OP.min)
                nc.sync.dma_start(out=out_g[r0:r0 + 128, :], in_=g)

            # band diagonals straight into DRAM (strided DRAM->DRAM copies)
            def diag_ap(dt, offset, count):
                return bass.AP(tensor=dt[:, :].tensor, offset=dt[:, :].offset + offset,
                               ap=[[S + 1, count]])

            nc.sync.dma_start(out=diag_ap(out_nb, 1, HALF), in_=dsup_d[1:1 + HALF])
            nc.sync.dma_start(out=diag_ap(out_nb, S, HALF - 1),
                              in_=dsup_d[1:HALF])
            nc.sync.dma_start(out=diag_ap(out_nb, 0, HALF), in_=dmain_d[0:HALF])
            nc.sync.dma_start(out=diag_ap(out_g, 0, HALF), in_=dmain_d[0:HALF])

    nc.compile()
    return nc


def _consts():
    k = np.arange(128)
    lt = (k[:, None] < k[None, :]).astype(np.float32)       # lt[k,p]=k<p
    import ml_dtypes
    ones = np.ones((128, 1), dtype=ml_dtypes.bfloat16)
    return lt, ones


def kernel(context, eos_mask, prior, wq, bq, wk, bk, gamma, beta):
    from concourse.bass_utils import run_bass_kernel_spmd

    if "nc" not in _cache:
        _cache["nc"] = _build()
    nc = _cache["nc"]

    import ml_dtypes
    bf = ml_dtypes.bfloat16
    context = np.asarray(context, np.float32).astype(bf)
    eos_mask = np.asarray(eos_mask, np.int32)
    prior = np.asarray(prior, np.float32)
    wq = np.asarray(wq, np.float32).astype(bf)
    wk = np.asarray(wk, np.float32).astype(bf)
    lt, ones = _consts()

    in_maps = []
    for c in range(8):
        b, h = c // 2, c % 2
        x = context[b] if h == 0 else context[b][::-1]
        eo = eos_mask[b] if h == 0 else eos_mask[b][::-1]
        eop = np.zeros(S + 2, np.int32)
        eop[1:S + 1] = eo
        in_maps.append({
            "x": np.ascontiguousarray(x),
            "eospad": eop,
            "prior": prior,
            "wq": wq, "wk": wk,
            "lt128": lt,
            "onesb": ones,
        })

    bkr = run_bass_kernel_spmd(nc, in_maps, core_ids=list(range(8)))
    _cache["last_bkr"] = bkr

    g_out = np.empty((B, S, S), np.float32)
    nb_out = np.empty((B, S, S), np.float32)
    for c in range(8):
        b, h = c // 2, c % 2
        rg = bkr.results[c]["out_g"]
        rn = bkr.results[c]["out_nb"]
        if h == 0:
            g_out[b, :HALF] = rg
            nb_out[b, :HALF] = rn
        else:
            g_out[b, HALF:] = rg[::-1, ::-1]
            nb_out[b, HALF:] = rn[::-1, ::-1]
    return g_out, nb_out
